# revision 2
# baseline (speedup 1.0000x reference)
"""Trainium2 Bass kernel for nn_MultiHeadAttention_25701084299319.

Reference computes (per batch b):
    q = x_b @ Wq ; k = x_b @ Wk ; v = x_b @ Wv
    y_b = sqrt(D) * (q @ k^T) @ v              (NO softmax)

Everything is linear, so reassociate to avoid the (T,T) attention matrix:
    y_b = g * x_b @ Wq @ Wk^T @ (x_b^T x_b) @ Wv
        = g * q_b @ R_b,   q_b = x_b Wq,  R_b = Wk^T (x_b^T x_b) Wv

FLOPs drop from 378 GF (naive attention) to ~97 GF total, with only DxD
intermediates.  Sharding: 8 cores = 4 batches x 2 row-halves. Each core
computes G_b = x_b^T x_b fully (duplicated within the pair; cheaper than
any collective), then its own half of the rows of y_b.

Per-core matmul dataflow (all fp32r = full-rate fp32 on the PE array,
contraction dim on partitions, lhsT convention out = lhsT.T @ rhs):
    A) qsT[d1, r]   = sum_d0 Wq[d0,d1] xsT[d0,r]         (lhsT=Wq,  rhs=xsT)
    B) G[d2, d3]    = sum_rho xb[rho,d2] xb[rho,d3]      (lhsT=xb,  rhs=xb)
    C) M[d2, d4]    = sum_d3 G[d3,d2] Wv[d3,d4]          (lhsT=G-symmetric, rhs=Wv)
    D) R[d1, d4]    = g * sum_d2 Wk[d2,d1] M[d2,d4]      (lhsT=Wk,  rhs=M)
    E) y[r, d4]     = sum_d1 qsT[d1,r] R[d1,d4]          (lhsT=qsT, rhs=R)
No on-chip transposes anywhere; the host supplies xsT (= x-half^T) as a
sharding/layout choice.
"""
import sys

if '/opt/trn_rl_repo' not in sys.path:
    sys.path.insert(0, '/opt/trn_rl_repo')

import numpy as np

B, T, D = 4, 4096, 1024
HALF = T // 2          # rows per core
N_CORES = 8
GAMMA = 32.0           # sqrt(D)

_NC_CACHE = {}


def build_nc(n_reps=1):
    import concourse.tile as tile
    from concourse import bacc, mybir

    F32 = mybir.dt.float32
    F32R = mybir.dt.float32r

    nc = bacc.Bacc("TRN2", target_bir_lowering=False, debug=False,
                   num_devices=N_CORES)
    xb = nc.dram_tensor("xb", [T, D], F32R, kind="ExternalInput").ap()
    xsT = nc.dram_tensor("xsT", [D, HALF], F32R, kind="ExternalInput").ap()
    wq = nc.dram_tensor("wq", [D, D], F32R, kind="ExternalInput").ap()
    wk = nc.dram_tensor("wk", [D, D], F32R, kind="ExternalInput").ap()
    wv = nc.dram_tensor("wv", [D, D], F32R, kind="ExternalInput").ap()
    y = nc.dram_tensor("y", [HALF, D], F32, kind="ExternalOutput").ap()

    KC = D // 128      # 8 contraction chunks of 128
    NQ = HALF // 512   # 4 r-quarters
    NR = HALF // 128   # 16 y row chunks
    XC = T // 128      # 32 xb row chunks

    def body(tc, dd, wtmp, stream, qsp, ps):
        nc = tc.nc
        # ---- loads ----
        wq_sb = [dd.tile([128, D], F32R, name=f"wq{k}", tag=f"dd{k}") for k in range(KC)]
        for k in range(KC):
            nc.scalar.dma_start(wq_sb[k][:], wq[k * 128:(k + 1) * 128, :])
        qsT_sb = [qsp.tile([128, HALF], F32R, name=f"qsT{m}", tag=f"q{m}") for m in range(KC)]

        # ---- A: qsT = Wq^T @ xsT  (per r-quarter: 8 psum banks) ----
        for q in range(NQ):
            xst_t = [stream.tile([128, 512], F32R, name=f"xst{q}_{k}", tag="stream") for k in range(KC)]
            for k in range(KC):
                nc.sync.dma_start(xst_t[k][:], xsT[k * 128:(k + 1) * 128, q * 512:(q + 1) * 512])
            accs = [ps.tile([128, 512], F32, name=f"psA{q}_{m}", tag="ps") for m in range(KC)]
            for k in range(KC):
                for m in range(KC):
                    nc.tensor.matmul(accs[m][:], wq_sb[k][:, m * 128:(m + 1) * 128],
                                     xst_t[k][:], start=(k == 0), stop=(k == KC - 1))
            for m in range(KC):
                nc.vector.tensor_copy(qsT_sb[m][:, q * 512:(q + 1) * 512], accs[m][:])

        # ---- B: G = xb^T @ xb  (two passes over xb, 8 psum banks held) ----
        G_sb = [dd.tile([128, D], F32R, name=f"G{m}", tag=f"dd{8 + m}") for m in range(KC)]
        for h in range(2):
            accs = [ps.tile([128, 512], F32, name=f"psB{h}_{m}", tag="ps") for m in range(KC)]
            for rho in range(XC):
                xc = stream.tile([128, D], F32R, name=f"xc{h}_{rho}", tag="stream")
                nc.sync.dma_start(xc[:], xb[rho * 128:(rho + 1) * 128, :])
                for m in range(KC):
                    nc.tensor.matmul(accs[m][:], xc[:, m * 128:(m + 1) * 128],
                                     xc[:, h * 512:(h + 1) * 512],
                                     start=(rho == 0), stop=(rho == XC - 1))
            for m in range(KC):
                nc.vector.tensor_copy(G_sb[m][:, h * 512:(h + 1) * 512], accs[m][:])

        # ---- C: M = G @ Wv  (G symmetric -> lhsT = G) ----
        wv_sb = [wtmp.tile([128, D], F32R, name=f"wv{k}", tag=f"wt{k}") for k in range(KC)]
        for k in range(KC):
            nc.scalar.dma_start(wv_sb[k][:], wv[k * 128:(k + 1) * 128, :])
        M_sb = [dd.tile([128, D], F32R, name=f"M{m}", tag=f"dd{m}") for m in range(KC)]
        for m in range(KC):
            accs = [ps.tile([128, 512], F32, name=f"psC{m}_{n}", tag="ps") for n in range(2)]
            for k in range(KC):
                for n in range(2):
                    nc.tensor.matmul(accs[n][:], G_sb[k][:, m * 128:(m + 1) * 128],
                                     wv_sb[k][:, n * 512:(n + 1) * 512],
                                     start=(k == 0), stop=(k == KC - 1))
            for n in range(2):
                nc.vector.tensor_copy(M_sb[m][:, n * 512:(n + 1) * 512], accs[n][:])

        # ---- D: R = gamma * Wk^T @ M ----
        wk_sb = [wtmp.tile([128, D], F32R, name=f"wk{k}", tag=f"wt{k}") for k in range(KC)]
        for k in range(KC):
            nc.scalar.dma_start(wk_sb[k][:], wk[k * 128:(k + 1) * 128, :])
        R_sb = [dd.tile([128, D], F32R, name=f"R{m}", tag=f"dd{8 + m}") for m in range(KC)]
        for m in range(KC):
            accs = [ps.tile([128, 512], F32, name=f"psD{m}_{n}", tag="ps") for n in range(2)]
            for k in range(KC):
                for n in range(2):
                    nc.tensor.matmul(accs[n][:], wk_sb[k][:, m * 128:(m + 1) * 128],
                                     M_sb[k][:, n * 512:(n + 1) * 512],
                                     start=(k == 0), stop=(k == KC - 1))
            for n in range(2):
                nc.vector.tensor_scalar_mul(R_sb[m][:, n * 512:(n + 1) * 512], accs[n][:], GAMMA)

        # ---- E: y = qsT^T @ R ----
        for mr in range(NR):
            accs = [ps.tile([128, 512], F32, name=f"psE{mr}_{n}", tag="ps") for n in range(2)]
            for k in range(KC):
                for n in range(2):
                    nc.tensor.matmul(accs[n][:], qsT_sb[k][:, mr * 128:(mr + 1) * 128],
                                     R_sb[k][:, n * 512:(n + 1) * 512],
                                     start=(k == 0), stop=(k == KC - 1))
            for n in range(2):
                yst = stream.tile([128, 512], F32, name=f"yst{mr}_{n}", tag="stream")
                nc.vector.tensor_copy(yst[:], accs[n][:])
                nc.sync.dma_start(y[mr * 128:(mr + 1) * 128, n * 512:(n + 1) * 512], yst[:])

    with tile.TileContext(nc) as tc:
        with tc.tile_pool(name="dd", bufs=1) as dd, \
             tc.tile_pool(name="wtmp", bufs=1) as wtmp, \
             tc.tile_pool(name="stream", bufs=6) as stream, \
             tc.tile_pool(name="qsp", bufs=1) as qsp, \
             tc.tile_pool(name="ps", bufs=8, space="PSUM") as ps:
            if n_reps == 1:
                body(tc, dd, wtmp, stream, qsp, ps)
            else:
                with tc.For_i(0, n_reps, 1):
                    body(tc, dd, wtmp, stream, qsp, ps)
    nc.compile()
    return nc


def get_nc(n_reps=1):
    if n_reps not in _NC_CACHE:
        _NC_CACHE[n_reps] = build_nc(n_reps)
    return _NC_CACHE[n_reps]


def make_in_maps(x, w_q, w_k, w_v):
    x = np.ascontiguousarray(x, dtype=np.float32)
    in_maps = []
    for c in range(N_CORES):
        b, h = c // 2, c % 2
        in_maps.append({
            "xb": x[b],
            "xsT": np.ascontiguousarray(x[b, h * HALF:(h + 1) * HALF, :].T),
            "wq": np.asarray(w_q, dtype=np.float32),
            "wk": np.asarray(w_k, dtype=np.float32),
            "wv": np.asarray(w_v, dtype=np.float32),
        })
    return in_maps


def gather(results):
    y = np.empty((B, T, D), dtype=np.float32)
    for c in range(N_CORES):
        b, h = c // 2, c % 2
        y[b, h * HALF:(h + 1) * HALF, :] = results[c]["y"]
    return y


def kernel(x, w_q, w_k, w_v):
    from concourse import bass_utils
    nc = get_nc()
    in_maps = make_in_maps(x, w_q, w_k, w_v)
    res = bass_utils.run_bass_kernel_spmd(nc, in_maps, core_ids=list(range(N_CORES)))
    return gather(res.results)
